# revision 20
# baseline (speedup 1.0000x reference)
"""CrossNet forward as a Trainium2 Bass/Tile kernel, data-parallel over 8 cores.

Math: the CrossNet layer stack
    x_{l+1} = x0 * (x_l . w_l) + b_l + x_l            (l = 0..3)
collapses in closed form.  Writing x_l = x0 * alpha_l[b] + beta_l[d]:
    p_l[b]     = sum_d x0[b,d] w_l[d]                 (4 projections of x0)
    alpha_0    = 1,   alpha_{l+1} = alpha_l * (1 + p_l) + c_l
    beta_{l+1} = beta_l + b_l,  c_l = beta_l . w_l    (host-computable scalars)
    out        = x0 * alpha_4[b] + beta_4[d]

Memory-bound problem: 16 MB fp16 in + 16 MB fp16 out per core at the
~358 GB/s HBM-per-NC limit gives a ~90 us floor.  v2 design notes:

- Host packs x to fp16 pair-interleaved supertiles [128, (j d q)] where
  row = s*1024 + p*8 + 2j+q, stored PARTITION-MAJOR in DRAM
  ([128, NSUP*512] f32 words) so multi-supertile loads are one long
  contiguous run per partition (8 KB descriptors instead of 2 KB).
- Per supertile: 4 packed fp32-dtype PE transposes (bit-exact 16-bit-halves
  routing, 2 fp16 chunks per transpose), ACT copies PSUM->SBUF, 8 fp16
  [128d,128b]^T @ [128d,4] projection matmuls on strided fp16 views, tiny
  f32 DVE recurrence for alpha with the last op emitting fp16.
- The final multiply keeps the OUTPUT in the same packed (j d q) layout so
  every operand (x, alpha-pair, out) is fp16 with innermost step +1: the
  alpha AP is the 32-bit pair [a1|a0] re-read with a stride-0 middle dim.
  That qualifies for the DVE 2x_1P dual-pump mode (mixed f32 broadcast ran
  1x and made stores trail loads by ~37 us in v1).
- fp16 store, host upcasts/unpacks.  Loads on the SP HWDGE ring, stores on
  the GpSimd SWDGE ring, consts on the ACT HWDGE ring so the first x load
  issues immediately.  The 36-row remainder runs FIRST so it does not
  serialize the kernel tail.  Final/store stage is software-pipelined one
  supertile behind the projection stage.
- float32r was measured to CORRUPT packed fp16 patterns on HW (rel err
  ~3.5) -- transposes must stay plain float32.
"""

import numpy as np

B = 500_000
D = 128
L = 4
N_CORES = 8
ROWS = B // N_CORES          # 62500 rows per core
G = 8                        # 128-row chunks per supertile
SUP = 128 * G                # 1024 rows per supertile
NSUP = ROWS // SUP           # 61 full supertiles
REM = ROWS - NSUP * SUP      # 36 remainder rows
NPAIR = G // 2               # 4 packed pairs per supertile
WPS = G * D // 2             # 512 fp32 words per partition per supertile

# Load group sizes: small groups first for fast pipeline ramp, then 1 MB
# transfers (8 KB per partition contiguous) for line-rate descriptors.
GROUPS = [1, 1, 2] + [4] * 14 + [1]
assert sum(GROUPS) == NSUP

TDT = "float32"

_CACHE: dict = {}

# test.py can read run metadata (exec_time_ns etc.) from here after a call.
LAST_RESULTS = None


def _build(cs, has_bias):
    import concourse.tile as tile
    from concourse import bacc, mybir

    f32 = mybir.dt.float32
    f16 = mybir.dt.float16
    tdt = getattr(mybir.dt, TDT)
    mult = mybir.AluOpType.mult
    add = mybir.AluOpType.add

    nc = bacc.Bacc(
        "TRN2",
        target_bir_lowering=False,
        debug=False,
        enable_asserts=False,
        num_devices=N_CORES,
    )
    # Partition-major packed input: word (p, s*512 + j*128 + d) holds the
    # fp16 pair [x(r1,d) | x(r0,d)] with r(q) = s*1024 + p*8 + 2j + q.
    xp = nc.dram_tensor("xp", [128, NSUP * WPS], tdt, kind="ExternalInput").ap()
    xrem = None
    if REM:
        xrem = nc.dram_tensor("xrem", [REM, D], f16, kind="ExternalInput").ap()
    w = nc.dram_tensor("w", [D, L], f16, kind="ExternalInput").ap()
    ident = nc.dram_tensor("ident", [128, 128], f16, kind="ExternalInput").ap()
    ident32 = nc.dram_tensor("ident32", [128, 128], tdt, kind="ExternalInput").ap()
    bb = None
    if has_bias:
        bb = nc.dram_tensor("bb", [128, 2 * D], f16, kind="ExternalInput").ap()
    # Output in the SAME packed (j d q) partition-major layout; host unpacks.
    out = nc.dram_tensor("out", [128, NSUP * G * D], f16, kind="ExternalOutput").ap()
    outr = None
    if REM:
        outr = nc.dram_tensor("outr", [REM, D], f16, kind="ExternalOutput").ap()

    with tile.TileContext(nc) as tc:
        with (
            tc.tile_pool(name="consts", bufs=1) as cpool,
            tc.tile_pool(name="xin", bufs=6) as xpool,
            tc.tile_pool(name="xt", bufs=8) as xtpool,
            tc.tile_pool(name="xtps", bufs=5, space="PSUM") as tps_pool,
            tc.tile_pool(name="ptps", bufs=2, space="PSUM") as pps_pool,
            tc.tile_pool(name="small", bufs=16) as spool,
            tc.tile_pool(name="outp", bufs=12) as opool,
        ):
            # Constants ride the ACT HWDGE ring so xp loads own the SP ring.
            # ident32 first: it gates the main pipeline's transposes.
            ident32_sb = cpool.tile([128, 128], tdt, tag="ident32")
            nc.scalar.dma_start(ident32_sb[:], ident32)
            w_sb = cpool.tile([D, L], f16, tag="w")
            nc.scalar.dma_start(w_sb[:], w)
            ident_sb = cpool.tile([128, 128], f16, tag="ident")
            nc.scalar.dma_start(ident_sb[:], ident)
            bb_sb = None
            if has_bias:
                bb_sb = cpool.tile([128, 2 * D], f16, tag="bb")
                nc.scalar.dma_start(bb_sb[:], bb)

            copy_fn = mybir.ActivationFunctionType.Copy

            def alpha_from_pt(pt_ps, p_cnt, g_cnt):
                # q = 1 + p on the ACT engine (reads PSUM, frees DVE cycles),
                # then product over the 4 layers; fp16 result.
                q_sb = spool.tile([p_cnt, L * g_cnt], f32, tag="q")
                nc.scalar.activation(q_sb[:], pt_ps[:], copy_fn, bias=1.0)
                a16 = spool.tile([p_cnt, g_cnt], f16, tag="a16")
                if has_bias:
                    qv = q_sb[:].rearrange("p (g l) -> p g l", l=L)
                    a = spool.tile([p_cnt, g_cnt], f32, tag="a0")
                    # c_0 == 0 always (beta_0 = 0)
                    nc.vector.tensor_copy(a[:], qv[:, :, 0])
                    for l in range(1, L):
                        last = l == L - 1
                        t = spool.tile([p_cnt, g_cnt], f32 if not last else f32,
                                       tag=f"a{l}")
                        nc.vector.tensor_mul(t[:], a[:], qv[:, :, l])
                        if cs[l] != 0.0:
                            t2 = spool.tile([p_cnt, g_cnt], f32, tag=f"ac{l}")
                            nc.vector.tensor_scalar_add(t2[:], t[:], float(cs[l]))
                            t = t2
                        a = t
                    nc.vector.tensor_copy(a16[:], a[:])
                else:
                    # t[g, u] = q[g, 2u] * q[g, 2u+1], then a = t[:,0]*t[:,1]
                    qp = q_sb[:].rearrange("p (g u l) -> p g u l", u=2, l=2)
                    t = spool.tile([p_cnt, 2 * g_cnt], f32, tag="a1")
                    tv = t[:].rearrange("p (g u) -> p g u", u=2)
                    nc.vector.tensor_mul(tv, qp[:, :, :, 0], qp[:, :, :, 1])
                    nc.vector.tensor_mul(a16[:], tv[:, :, 0], tv[:, :, 1])
                return a16

            def project_st(xp32, pt_slice):
                # xp32: [128, 512] f32 view of this supertile's packed words.
                # --- packed PE transposes (2 fp16 chunks per transpose) ---
                # All 4 pairs land in ONE PSUM bank; one ACT copy per
                # supertile.  Cross-supertile overlap comes from tps bufs.
                xt_ps = tps_pool.tile([128, NPAIR * D], tdt, tag="xtps")
                xt_sb = xtpool.tile([128, NPAIR * D], tdt, tag="xt")
                for j in range(NPAIR):
                    nc.tensor.transpose(
                        xt_ps[:, j * D : (j + 1) * D],
                        xp32[:, j * D : (j + 1) * D],
                        ident32_sb[:],
                    )
                nc.scalar.copy(xt_sb[:], xt_ps[:])

                # --- projection matmuls: pt[:, g*L:(g+1)*L], g = 2j+q ---
                xt16 = xt_sb[:].bitcast(f16).rearrange("d (j b q) -> d j b q", b=D, q=2)
                for g in range(G):
                    j, qq = g // 2, g % 2
                    nc.tensor.matmul(
                        pt_slice[:, g * L : (g + 1) * L],
                        lhsT=xt16[:, j, :, qq],
                        rhs=w_sb[:],
                        start=True,
                        stop=True,
                    )

            NB = 4  # supertiles per store batch (1 MB stores)
            pair_tile = [None]

            def emit_final(s, xp32, a16):
                # Packed-layout multiply: every operand fp16 innermost step 1
                # -> DVE dual-pump.  alpha pair [a1|a0] re-read across d.
                # NB supertiles share one output tile so stores are 1 MB
                # (8 KB/partition descriptors, 1/NB the SWDGE descgen work).
                half = s % NB
                if half == 0:
                    pair_tile[0] = opool.tile(
                        [128, NB * G * D], f16, name="opair", tag="o"
                    )
                out_sb = pair_tile[0]
                base = half * G * D
                ov4 = out_sb[:, base : base + G * D].rearrange(
                    "p (j d q) -> p j d q", d=D, q=2
                )
                xv4 = xp32.bitcast(f16).rearrange("p (j d q) -> p j d q", d=D, q=2)
                av4 = (
                    a16[:]
                    .rearrange("p (j q) -> p j q", q=2)
                    .unsqueeze(2)
                    .broadcast_to([128, NPAIR, D, 2])
                )
                if has_bias:
                    t_sb = opool.tile([128, G * D], f16, tag="t")
                    tv4 = t_sb[:].rearrange("p (j d q) -> p j d q", d=D, q=2)
                    bv4 = (
                        bb_sb[:]
                        .rearrange("p (d q) -> p d q", q=2)
                        .unsqueeze(1)
                        .broadcast_to([128, NPAIR, D, 2])
                    )
                    nc.vector.tensor_mul(tv4, xv4, av4)
                    nc.vector.tensor_add(ov4, tv4, bv4)
                else:
                    nc.vector.tensor_mul(ov4, xv4, av4)
                if half == NB - 1 or s == NSUP - 1:
                    n = (half + 1) * G * D
                    s0 = s - half
                    nc.gpsimd.dma_start(
                        out[:, s0 * G * D : s0 * G * D + n], out_sb[:, :n]
                    )

            def block_rem():
                p_cnt = REM
                x_sb = xpool.tile([p_cnt, D], f16, tag="xr")
                # scalar ring: keep the SP ring exclusively for x loads
                nc.scalar.dma_start(x_sb[:], xrem)
                xt_ps = tps_pool.tile([128, p_cnt], f16, tag="xtps")
                xt_sb = xtpool.tile([128, p_cnt], f16, tag="xtr")
                pt_ps = pps_pool.tile([p_cnt, L], f32, tag="pt")
                nc.tensor.transpose(xt_ps[:], x_sb[:], ident_sb[:p_cnt, :p_cnt])
                nc.scalar.copy(xt_sb[:], xt_ps[:])
                nc.tensor.matmul(
                    pt_ps[:], lhsT=xt_sb[:], rhs=w_sb[:], start=True, stop=True
                )
                a16 = alpha_from_pt(pt_ps, p_cnt, 1)
                out_sb = opool.tile([p_cnt, D], f16, tag="or")
                if has_bias:
                    t_sb = opool.tile([p_cnt, D], f16, tag="tr")
                    nc.vector.tensor_mul(
                        t_sb[:].rearrange("p (u d) -> p u d", u=1),
                        x_sb[:].rearrange("p (u d) -> p u d", u=1),
                        a16[:].to_broadcast([p_cnt, 1, D]),
                    )
                    bv = bb_sb[:p_cnt].rearrange("p (d q) -> p d q", q=2)[:, :, 0]
                    nc.vector.tensor_add(out_sb[:], t_sb[:], bv)
                else:
                    nc.vector.tensor_mul(
                        out_sb[:].rearrange("p (u d) -> p u d", u=1),
                        x_sb[:].rearrange("p (u d) -> p u d", u=1),
                        a16[:].to_broadcast([p_cnt, 1, D]),
                    )
                # HWDGE: the SWDGE ring serializes this tiny 36-partition
                # store behind every queued 1 MB store; sync ring has slack.
                nc.sync.dma_start(outr, out_sb[:])

            # Projections run per supertile; the alpha chain is batched over
            # PAIRS of supertiles (one pt tile, one ACT +1, two DVE muls) to
            # halve DVE small-op overhead.  Finished (s, x, alpha) tuples
            # queue in `flights`; emits trail by ~2 supertiles so the DVE
            # multiply/store cadence stays smooth.
            pair = []     # [(s, xp32), ...] accumulating to 2
            flights = []  # [(s, xp32, a16slice), ...] awaiting emit

            def flush_pair():
                if not pair:
                    return
                n = len(pair)
                pt = pps_pool.tile([128, n * L * G], f32, name="pt", tag="pt")
                for i, (ss, xv) in enumerate(pair):
                    project_st(xv, pt[:, i * L * G : (i + 1) * L * G])
                a16 = alpha_from_pt(pt, 128, n * G)
                for i, (ss, xv) in enumerate(pair):
                    flights.append((ss, xv, a16[:, i * G : (i + 1) * G]))
                pair.clear()

            s = 0
            rem_done = not REM
            for gsz in GROUPS:
                gt = xpool.tile([128, gsz * WPS], tdt, tag="x")
                nc.sync.dma_start(gt[:], xp[:, s * WPS : (s + gsz) * WPS])
                for ls in range(gsz):
                    pair.append((s + ls, gt[:, ls * WPS : (ls + 1) * WPS]))
                    if len(pair) == 2:
                        flush_pair()
                    while len(flights) > 2:
                        emit_final(*flights.pop(0))
                s += gsz
                if not rem_done and s >= 4:
                    # Mid-stream: independent work, fills scheduling slack
                    # without delaying the first loads or the kernel tail.
                    block_rem()
                    rem_done = True
            flush_pair()
            while flights:
                emit_final(*flights.pop(0))

    nc.compile()
    return nc


def _pack_shard(xs):
    # xs: [ROWS, D] float32 -> fp16 packed partition-major [128, NSUP*512] f32
    # words; word (p, s, j, d) = [x(s*1024+p*8+2j+1, d) | x(s*1024+p*8+2j, d)].
    x16 = xs[: NSUP * SUP].astype(np.float16).reshape(NSUP, 128, NPAIR, 2, D)
    # -> (p, s, j, d, q)
    pk = np.ascontiguousarray(x16.transpose(1, 0, 2, 4, 3)).reshape(128, -1)
    return pk.view(np.float32)


def _unpack_out(o, orem):
    # o: [128, NSUP*1024] f16 packed (s, j, d, q) per partition -> [ROWS, D] f32
    o5 = o.reshape(128, NSUP, NPAIR, D, 2).transpose(1, 0, 2, 4, 3)
    main = np.ascontiguousarray(o5).reshape(NSUP * SUP, D)
    full = np.empty((ROWS, D), dtype=np.float32)
    full[: NSUP * SUP] = main
    if REM:
        full[NSUP * SUP :] = orem
    return full


def kernel(inputs, kernels, biases):
    global LAST_RESULTS
    import os

    if os.environ.get("BASS_TRACE"):
        # run_bass_kernel_spmd's trace path hard-imports antenv.axon_hooks,
        # which not every image ships; fall back to no-trace instead of
        # crashing when it is absent.
        try:
            import antenv.axon_hooks  # noqa: F401
        except ImportError:
            os.environ["BASS_NEVER_TRACE"] = "1"

    from concourse.bass_utils import run_bass_kernel_spmd

    x = np.ascontiguousarray(np.asarray(inputs), dtype=np.float32)
    assert x.shape == (B, D), x.shape
    kern = np.asarray(kernels, dtype=np.float32).reshape(L, D)
    bias = np.asarray(biases, dtype=np.float32).reshape(L, D)

    W = np.ascontiguousarray(kern.T)  # [D, L]
    has_bias = bool(np.any(bias))
    cs = []
    beta = np.zeros(D, dtype=np.float32)
    for l in range(L):
        cs.append(float(np.dot(beta.astype(np.float64), kern[l].astype(np.float64))))
        beta = beta + bias[l]

    key = (has_bias, tuple(cs) if has_bias else None)
    nc = _CACHE.get(key)
    if nc is None:
        nc = _build(cs, has_bias)
        _CACHE[key] = nc

    in_maps = []
    for i in range(N_CORES):
        xs = x[i * ROWS : (i + 1) * ROWS]
        m = {
            "xp": _pack_shard(xs),
            "w": W.astype(np.float16),
            "ident": np.eye(128, dtype=np.float16),
            "ident32": np.eye(128, dtype=np.float32),
        }
        if REM:
            m["xrem"] = xs[NSUP * SUP :].astype(np.float16)
        if has_bias:
            # bb[p, (d q)] = beta[d] for both q halves.
            bb16 = np.broadcast_to(
                beta.astype(np.float16)[None, :, None], (128, D, 2)
            )
            m["bb"] = np.ascontiguousarray(bb16).reshape(128, 2 * D)
        in_maps.append(m)

    res = run_bass_kernel_spmd(nc, in_maps, core_ids=list(range(N_CORES)))
    LAST_RESULTS = res
    outs = []
    for i in range(N_CORES):
        o = res.results[i]["out"]
        orem = res.results[i]["outr"] if REM else None
        outs.append(_unpack_out(o, orem))
    return np.concatenate(outs, axis=0).astype(np.float32)


# revision 22
# speedup vs baseline: 1.0613x; 1.0613x over previous
"""CrossNet forward as a Trainium2 Bass/Tile kernel, data-parallel over 8 cores.

Math: the CrossNet layer stack
    x_{l+1} = x0 * (x_l . w_l) + b_l + x_l            (l = 0..3)
collapses in closed form.  Writing x_l = x0 * alpha_l[b] + beta_l[d]:
    p_l[b]     = sum_d x0[b,d] w_l[d]                 (4 projections of x0)
    alpha_0    = 1,   alpha_{l+1} = alpha_l * (1 + p_l) + c_l
    beta_{l+1} = beta_l + b_l,  c_l = beta_l . w_l    (host-computable scalars)
    out        = x0 * alpha_4[b] + beta_4[d]

Memory-bound problem: 16 MB fp16 in + 16 MB fp16 out per core at the
~358 GB/s HBM-per-NC limit gives a ~90 us floor.  v2 design notes:

- Host packs x to fp16 pair-interleaved supertiles [128, (j d q)] where
  row = s*1024 + p*8 + 2j+q, stored PARTITION-MAJOR in DRAM
  ([128, NSUP*512] f32 words) so multi-supertile loads are one long
  contiguous run per partition (8 KB descriptors instead of 2 KB).
- Per supertile: 4 packed fp32-dtype PE transposes (bit-exact 16-bit-halves
  routing, 2 fp16 chunks per transpose), ACT copies PSUM->SBUF, 8 fp16
  [128d,128b]^T @ [128d,4] projection matmuls on strided fp16 views, tiny
  f32 DVE recurrence for alpha with the last op emitting fp16.
- The final multiply keeps the OUTPUT in the same packed (j d q) layout so
  every operand (x, alpha-pair, out) is fp16 with innermost step +1: the
  alpha AP is the 32-bit pair [a1|a0] re-read with a stride-0 middle dim.
  That qualifies for the DVE 2x_1P dual-pump mode (mixed f32 broadcast ran
  1x and made stores trail loads by ~37 us in v1).
- fp16 store, host upcasts/unpacks.  Loads on the SP HWDGE ring, stores on
  the GpSimd SWDGE ring, consts on the ACT HWDGE ring so the first x load
  issues immediately.  The 36-row remainder runs FIRST so it does not
  serialize the kernel tail.  Final/store stage is software-pipelined one
  supertile behind the projection stage.
- float32r was measured to CORRUPT packed fp16 patterns on HW (rel err
  ~3.5) -- transposes must stay plain float32.
"""

import numpy as np

B = 500_000
D = 128
L = 4
N_CORES = 8
ROWS = B // N_CORES          # 62500 rows per core
G = 8                        # 128-row chunks per supertile
SUP = 128 * G                # 1024 rows per supertile
NSUP = ROWS // SUP           # 61 full supertiles
REM = ROWS - NSUP * SUP      # 36 remainder rows
NPAIR = G // 2               # 4 packed pairs per supertile
WPS = G * D // 2             # 512 fp32 words per partition per supertile

# Load group sizes: small groups first for fast pipeline ramp, then 1 MB
# transfers (8 KB per partition contiguous) for line-rate descriptors.
GROUPS = [1, 1, 2] + [4] * 14 + [1]
assert sum(GROUPS) == NSUP

TDT = "float32"

_CACHE: dict = {}

# test.py can read run metadata (exec_time_ns etc.) from here after a call.
LAST_RESULTS = None


def _build(cs, has_bias):
    import concourse.tile as tile
    from concourse import bacc, mybir

    f32 = mybir.dt.float32
    f16 = mybir.dt.float16
    tdt = getattr(mybir.dt, TDT)
    mult = mybir.AluOpType.mult
    add = mybir.AluOpType.add

    nc = bacc.Bacc(
        "TRN2",
        target_bir_lowering=False,
        debug=False,
        enable_asserts=False,
        num_devices=N_CORES,
    )
    # Partition-major packed input: word (p, s*512 + j*128 + d) holds the
    # fp16 pair [x(r1,d) | x(r0,d)] with r(q) = s*1024 + p*8 + 2j + q.
    xp = nc.dram_tensor("xp", [128, NSUP * WPS], tdt, kind="ExternalInput").ap()
    xrem = None
    if REM:
        xrem = nc.dram_tensor("xrem", [REM, D], f16, kind="ExternalInput").ap()
    w = nc.dram_tensor("w", [D, L], f16, kind="ExternalInput").ap()
    ident = nc.dram_tensor("ident", [128, 128], f16, kind="ExternalInput").ap()
    ident32 = nc.dram_tensor("ident32", [128, 128], tdt, kind="ExternalInput").ap()
    bb = None
    if has_bias:
        bb = nc.dram_tensor("bb", [128, 2 * D], f16, kind="ExternalInput").ap()
    # Output in the SAME packed (j d q) partition-major layout; host unpacks.
    out = nc.dram_tensor("out", [128, NSUP * G * D], f16, kind="ExternalOutput").ap()
    outr = None
    if REM:
        outr = nc.dram_tensor("outr", [REM, D], f16, kind="ExternalOutput").ap()

    with tile.TileContext(nc) as tc:
        with (
            tc.tile_pool(name="consts", bufs=1) as cpool,
            tc.tile_pool(name="xin", bufs=6) as xpool,
            tc.tile_pool(name="xt", bufs=8) as xtpool,
            tc.tile_pool(name="xtps", bufs=5, space="PSUM") as tps_pool,
            tc.tile_pool(name="ptps", bufs=2, space="PSUM") as pps_pool,
            tc.tile_pool(name="small", bufs=16) as spool,
            tc.tile_pool(name="outp", bufs=12) as opool,
        ):
            # Constants ride the ACT HWDGE ring so xp loads own the SP ring.
            # ident32 first: it gates the main pipeline's transposes.
            ident32_sb = cpool.tile([128, 128], tdt, tag="ident32")
            nc.scalar.dma_start(ident32_sb[:], ident32)
            w_sb = cpool.tile([D, L], f16, tag="w")
            nc.scalar.dma_start(w_sb[:], w)
            ident_sb = cpool.tile([128, 128], f16, tag="ident")
            nc.scalar.dma_start(ident_sb[:], ident)
            bb_sb = None
            if has_bias:
                bb_sb = cpool.tile([128, 2 * D], f16, tag="bb")
                nc.scalar.dma_start(bb_sb[:], bb)

            copy_fn = mybir.ActivationFunctionType.Copy

            def alpha_from_pt(pt_ps, p_cnt, g_cnt):
                # q = 1 + p on the ACT engine (reads PSUM, frees DVE cycles),
                # then product over the 4 layers; fp16 result.
                q_sb = spool.tile([p_cnt, L * g_cnt], f32, tag="q")
                nc.scalar.activation(q_sb[:], pt_ps[:], copy_fn, bias=1.0)
                a16 = spool.tile([p_cnt, g_cnt], f16, tag="a16")
                if has_bias:
                    qv = q_sb[:].rearrange("p (g l) -> p g l", l=L)
                    a = spool.tile([p_cnt, g_cnt], f32, tag="a0")
                    # c_0 == 0 always (beta_0 = 0)
                    nc.vector.tensor_copy(a[:], qv[:, :, 0])
                    for l in range(1, L):
                        last = l == L - 1
                        t = spool.tile([p_cnt, g_cnt], f32 if not last else f32,
                                       tag=f"a{l}")
                        nc.vector.tensor_mul(t[:], a[:], qv[:, :, l])
                        if cs[l] != 0.0:
                            t2 = spool.tile([p_cnt, g_cnt], f32, tag=f"ac{l}")
                            nc.vector.tensor_scalar_add(t2[:], t[:], float(cs[l]))
                            t = t2
                        a = t
                    nc.vector.tensor_copy(a16[:], a[:])
                else:
                    # t[g, u] = q[g, 2u] * q[g, 2u+1], then a = t[:,0]*t[:,1]
                    qp = q_sb[:].rearrange("p (g u l) -> p g u l", u=2, l=2)
                    t = spool.tile([p_cnt, 2 * g_cnt], f32, tag="a1")
                    tv = t[:].rearrange("p (g u) -> p g u", u=2)
                    nc.vector.tensor_mul(tv, qp[:, :, :, 0], qp[:, :, :, 1])
                    nc.vector.tensor_mul(a16[:], tv[:, :, 0], tv[:, :, 1])
                return a16

            def project_st(xp32, pt_slice):
                # xp32: [128, 512] f32 view of this supertile's packed words.
                # --- packed PE transposes (2 fp16 chunks per transpose) ---
                # All 4 pairs land in ONE PSUM bank; one ACT copy per
                # supertile.  Cross-supertile overlap comes from tps bufs.
                xt_ps = tps_pool.tile([128, NPAIR * D], tdt, tag="xtps")
                xt_sb = xtpool.tile([128, NPAIR * D], tdt, tag="xt")
                for j in range(NPAIR):
                    nc.tensor.transpose(
                        xt_ps[:, j * D : (j + 1) * D],
                        xp32[:, j * D : (j + 1) * D],
                        ident32_sb[:],
                    )
                nc.scalar.copy(xt_sb[:], xt_ps[:])

                # --- projection matmuls: pt[:, g*L:(g+1)*L], g = 2j+q ---
                xt16 = xt_sb[:].bitcast(f16).rearrange("d (j b q) -> d j b q", b=D, q=2)
                for g in range(G):
                    j, qq = g // 2, g % 2
                    nc.tensor.matmul(
                        pt_slice[:, g * L : (g + 1) * L],
                        lhsT=xt16[:, j, :, qq],
                        rhs=w_sb[:],
                        start=True,
                        stop=True,
                    )

            NB = 4  # supertiles per store batch (1 MB stores)
            pair_tile = [None]

            def emit_final(s, xp32, a16):
                # Packed-layout multiply: every operand fp16 innermost step 1
                # -> DVE dual-pump.  alpha pair [a1|a0] re-read across d.
                # NB supertiles share one output tile so stores are 1 MB
                # (8 KB/partition descriptors, 1/NB the SWDGE descgen work).
                half = s % NB
                if half == 0:
                    pair_tile[0] = opool.tile(
                        [128, NB * G * D], f16, name="opair", tag="o"
                    )
                out_sb = pair_tile[0]
                base = half * G * D
                ov4 = out_sb[:, base : base + G * D].rearrange(
                    "p (j d q) -> p j d q", d=D, q=2
                )
                xv4 = xp32.bitcast(f16).rearrange("p (j d q) -> p j d q", d=D, q=2)
                av4 = (
                    a16[:]
                    .rearrange("p (j q) -> p j q", q=2)
                    .unsqueeze(2)
                    .broadcast_to([128, NPAIR, D, 2])
                )
                if has_bias:
                    t_sb = opool.tile([128, G * D], f16, tag="t")
                    tv4 = t_sb[:].rearrange("p (j d q) -> p j d q", d=D, q=2)
                    bv4 = (
                        bb_sb[:]
                        .rearrange("p (d q) -> p d q", q=2)
                        .unsqueeze(1)
                        .broadcast_to([128, NPAIR, D, 2])
                    )
                    nc.vector.tensor_mul(tv4, xv4, av4)
                    nc.vector.tensor_add(ov4, tv4, bv4)
                else:
                    nc.vector.tensor_mul(ov4, xv4, av4)
                if half == NB - 1 or s == NSUP - 1:
                    n = (half + 1) * G * D
                    s0 = s - half
                    nc.gpsimd.dma_start(
                        out[:, s0 * G * D : s0 * G * D + n], out_sb[:, :n]
                    )

            def block_rem():
                p_cnt = REM
                x_sb = xpool.tile([p_cnt, D], f16, tag="xr")
                # scalar ring: keep the SP ring exclusively for x loads
                nc.scalar.dma_start(x_sb[:], xrem)
                xt_ps = tps_pool.tile([128, p_cnt], f16, tag="xtps")
                xt_sb = xtpool.tile([128, p_cnt], f16, tag="xtr")
                pt_ps = pps_pool.tile([p_cnt, L], f32, tag="pt")
                nc.tensor.transpose(xt_ps[:], x_sb[:], ident_sb[:p_cnt, :p_cnt])
                nc.scalar.copy(xt_sb[:], xt_ps[:])
                nc.tensor.matmul(
                    pt_ps[:], lhsT=xt_sb[:], rhs=w_sb[:], start=True, stop=True
                )
                a16 = alpha_from_pt(pt_ps, p_cnt, 1)
                out_sb = opool.tile([p_cnt, D], f16, tag="or")
                if has_bias:
                    t_sb = opool.tile([p_cnt, D], f16, tag="tr")
                    nc.vector.tensor_mul(
                        t_sb[:].rearrange("p (u d) -> p u d", u=1),
                        x_sb[:].rearrange("p (u d) -> p u d", u=1),
                        a16[:].to_broadcast([p_cnt, 1, D]),
                    )
                    bv = bb_sb[:p_cnt].rearrange("p (d q) -> p d q", q=2)[:, :, 0]
                    nc.vector.tensor_add(out_sb[:], t_sb[:], bv)
                else:
                    nc.vector.tensor_mul(
                        out_sb[:].rearrange("p (u d) -> p u d", u=1),
                        x_sb[:].rearrange("p (u d) -> p u d", u=1),
                        a16[:].to_broadcast([p_cnt, 1, D]),
                    )
                # SWDGE: a sync-ring store would head-of-line block the
                # group loads behind it in the HWDGE FIFO (measured +4.9 us).
                nc.gpsimd.dma_start(outr, out_sb[:])

            # Projections run per supertile; the alpha chain is batched over
            # PAIRS of supertiles (one pt tile, one ACT +1, two DVE muls) to
            # halve DVE small-op overhead.  Finished (s, x, alpha) tuples
            # queue in `flights`; emits trail by ~2 supertiles so the DVE
            # multiply/store cadence stays smooth.
            pair = []     # [(s, xp32), ...] accumulating to 2
            flights = []  # [(s, xp32, a16slice), ...] awaiting emit

            def flush_pair():
                if not pair:
                    return
                n = len(pair)
                pt = pps_pool.tile([128, n * L * G], f32, name="pt", tag="pt")
                for i, (ss, xv) in enumerate(pair):
                    project_st(xv, pt[:, i * L * G : (i + 1) * L * G])
                a16 = alpha_from_pt(pt, 128, n * G)
                for i, (ss, xv) in enumerate(pair):
                    flights.append((ss, xv, a16[:, i * G : (i + 1) * G]))
                pair.clear()

            s = 0
            rem_done = not REM
            for gsz in GROUPS:
                gt = xpool.tile([128, gsz * WPS], tdt, tag="x")
                nc.sync.dma_start(gt[:], xp[:, s * WPS : (s + gsz) * WPS])
                for ls in range(gsz):
                    pair.append((s + ls, gt[:, ls * WPS : (ls + 1) * WPS]))
                    if len(pair) == 2:
                        flush_pair()
                    while len(flights) > 2:
                        emit_final(*flights.pop(0))
                s += gsz
                if not rem_done and s >= 8:
                    # Mid-stream: independent work, fills scheduling slack
                    # without delaying the first loads or the kernel tail.
                    block_rem()
                    rem_done = True
            flush_pair()
            while flights:
                emit_final(*flights.pop(0))

    nc.compile()
    return nc


def _pack_shard(xs):
    # xs: [ROWS, D] float32 -> fp16 packed partition-major [128, NSUP*512] f32
    # words; word (p, s, j, d) = [x(s*1024+p*8+2j+1, d) | x(s*1024+p*8+2j, d)].
    x16 = xs[: NSUP * SUP].astype(np.float16).reshape(NSUP, 128, NPAIR, 2, D)
    # -> (p, s, j, d, q)
    pk = np.ascontiguousarray(x16.transpose(1, 0, 2, 4, 3)).reshape(128, -1)
    return pk.view(np.float32)


def _unpack_out(o, orem):
    # o: [128, NSUP*1024] f16 packed (s, j, d, q) per partition -> [ROWS, D] f32
    o5 = o.reshape(128, NSUP, NPAIR, D, 2).transpose(1, 0, 2, 4, 3)
    main = np.ascontiguousarray(o5).reshape(NSUP * SUP, D)
    full = np.empty((ROWS, D), dtype=np.float32)
    full[: NSUP * SUP] = main
    if REM:
        full[NSUP * SUP :] = orem
    return full


def kernel(inputs, kernels, biases):
    global LAST_RESULTS
    import os

    if os.environ.get("BASS_TRACE"):
        # run_bass_kernel_spmd's trace path hard-imports antenv.axon_hooks,
        # which not every image ships; fall back to no-trace instead of
        # crashing when it is absent.
        try:
            import antenv.axon_hooks  # noqa: F401
        except ImportError:
            os.environ["BASS_NEVER_TRACE"] = "1"

    from concourse.bass_utils import run_bass_kernel_spmd

    x = np.ascontiguousarray(np.asarray(inputs), dtype=np.float32)
    assert x.shape == (B, D), x.shape
    kern = np.asarray(kernels, dtype=np.float32).reshape(L, D)
    bias = np.asarray(biases, dtype=np.float32).reshape(L, D)

    W = np.ascontiguousarray(kern.T)  # [D, L]
    has_bias = bool(np.any(bias))
    cs = []
    beta = np.zeros(D, dtype=np.float32)
    for l in range(L):
        cs.append(float(np.dot(beta.astype(np.float64), kern[l].astype(np.float64))))
        beta = beta + bias[l]

    key = (has_bias, tuple(cs) if has_bias else None)
    nc = _CACHE.get(key)
    if nc is None:
        nc = _build(cs, has_bias)
        _CACHE[key] = nc

    in_maps = []
    for i in range(N_CORES):
        xs = x[i * ROWS : (i + 1) * ROWS]
        m = {
            "xp": _pack_shard(xs),
            "w": W.astype(np.float16),
            "ident": np.eye(128, dtype=np.float16),
            "ident32": np.eye(128, dtype=np.float32),
        }
        if REM:
            m["xrem"] = xs[NSUP * SUP :].astype(np.float16)
        if has_bias:
            # bb[p, (d q)] = beta[d] for both q halves.
            bb16 = np.broadcast_to(
                beta.astype(np.float16)[None, :, None], (128, D, 2)
            )
            m["bb"] = np.ascontiguousarray(bb16).reshape(128, 2 * D)
        in_maps.append(m)

    res = run_bass_kernel_spmd(nc, in_maps, core_ids=list(range(N_CORES)))
    LAST_RESULTS = res
    outs = []
    for i in range(N_CORES):
        o = res.results[i]["out"]
        orem = res.results[i]["outr"] if REM else None
        outs.append(_unpack_out(o, orem))
    return np.concatenate(outs, axis=0).astype(np.float32)


# revision 23
# speedup vs baseline: 1.0687x; 1.0070x over previous
"""CrossNet forward as a Trainium2 Bass/Tile kernel, data-parallel over 8 cores.

Math: the CrossNet layer stack
    x_{l+1} = x0 * (x_l . w_l) + b_l + x_l            (l = 0..3)
collapses in closed form.  Writing x_l = x0 * alpha_l[b] + beta_l[d]:
    p_l[b]     = sum_d x0[b,d] w_l[d]                 (4 projections of x0)
    alpha_0    = 1,   alpha_{l+1} = alpha_l * (1 + p_l) + c_l
    beta_{l+1} = beta_l + b_l,  c_l = beta_l . w_l    (host-computable scalars)
    out        = x0 * alpha_4[b] + beta_4[d]

Memory-bound problem: 16 MB fp16 in + 16 MB fp16 out per core at the
~358 GB/s HBM-per-NC limit gives a ~90 us floor.  v2 design notes:

- Host packs x to fp16 pair-interleaved supertiles [128, (j d q)] where
  row = s*1024 + p*8 + 2j+q, stored PARTITION-MAJOR in DRAM
  ([128, NSUP*512] f32 words) so multi-supertile loads are one long
  contiguous run per partition (8 KB descriptors instead of 2 KB).
- Per supertile: 4 packed fp32-dtype PE transposes (bit-exact 16-bit-halves
  routing, 2 fp16 chunks per transpose), ACT copies PSUM->SBUF, 8 fp16
  [128d,128b]^T @ [128d,4] projection matmuls on strided fp16 views, tiny
  f32 DVE recurrence for alpha with the last op emitting fp16.
- The final multiply keeps the OUTPUT in the same packed (j d q) layout so
  every operand (x, alpha-pair, out) is fp16 with innermost step +1: the
  alpha AP is the 32-bit pair [a1|a0] re-read with a stride-0 middle dim.
  That qualifies for the DVE 2x_1P dual-pump mode (mixed f32 broadcast ran
  1x and made stores trail loads by ~37 us in v1).
- fp16 store, host upcasts/unpacks.  Loads on the SP HWDGE ring, stores on
  the GpSimd SWDGE ring, consts on the ACT HWDGE ring so the first x load
  issues immediately.  The 36-row remainder runs FIRST so it does not
  serialize the kernel tail.  Final/store stage is software-pipelined one
  supertile behind the projection stage.
- float32r was measured to CORRUPT packed fp16 patterns on HW (rel err
  ~3.5) -- transposes must stay plain float32.
"""

import numpy as np

B = 500_000
D = 128
L = 4
N_CORES = 8
ROWS = B // N_CORES          # 62500 rows per core
G = 8                        # 128-row chunks per supertile
SUP = 128 * G                # 1024 rows per supertile
NSUP = ROWS // SUP           # 61 full supertiles
REM = ROWS - NSUP * SUP      # 36 remainder rows
NPAIR = G // 2               # 4 packed pairs per supertile
WPS = G * D // 2             # 512 fp32 words per partition per supertile

# Load group sizes: small groups first for fast pipeline ramp, then 1 MB
# transfers (8 KB per partition contiguous) for line-rate descriptors.
GROUPS = [1, 1, 2] + [4] * 13 + [2, 2, 1]
assert sum(GROUPS) == NSUP

TDT = "float32"

_CACHE: dict = {}

# test.py can read run metadata (exec_time_ns etc.) from here after a call.
LAST_RESULTS = None


def _build(cs, has_bias):
    import concourse.tile as tile
    from concourse import bacc, mybir

    f32 = mybir.dt.float32
    f16 = mybir.dt.float16
    tdt = getattr(mybir.dt, TDT)
    mult = mybir.AluOpType.mult
    add = mybir.AluOpType.add

    nc = bacc.Bacc(
        "TRN2",
        target_bir_lowering=False,
        debug=False,
        enable_asserts=False,
        num_devices=N_CORES,
    )
    # Partition-major packed input: word (p, s*512 + j*128 + d) holds the
    # fp16 pair [x(r1,d) | x(r0,d)] with r(q) = s*1024 + p*8 + 2j + q.
    xp = nc.dram_tensor("xp", [128, NSUP * WPS], tdt, kind="ExternalInput").ap()
    xrem = None
    if REM:
        xrem = nc.dram_tensor("xrem", [REM, D], f16, kind="ExternalInput").ap()
    w = nc.dram_tensor("w", [D, L], f16, kind="ExternalInput").ap()
    ident = nc.dram_tensor("ident", [128, 128], f16, kind="ExternalInput").ap()
    ident32 = nc.dram_tensor("ident32", [128, 128], tdt, kind="ExternalInput").ap()
    bb = None
    if has_bias:
        bb = nc.dram_tensor("bb", [128, 2 * D], f16, kind="ExternalInput").ap()
    # Output in the SAME packed (j d q) partition-major layout; host unpacks.
    out = nc.dram_tensor("out", [128, NSUP * G * D], f16, kind="ExternalOutput").ap()
    outr = None
    if REM:
        outr = nc.dram_tensor("outr", [REM, D], f16, kind="ExternalOutput").ap()

    with tile.TileContext(nc) as tc:
        with (
            tc.tile_pool(name="consts", bufs=1) as cpool,
            tc.tile_pool(name="xin", bufs=6) as xpool,
            tc.tile_pool(name="xt", bufs=8) as xtpool,
            tc.tile_pool(name="xtps", bufs=5, space="PSUM") as tps_pool,
            tc.tile_pool(name="ptps", bufs=2, space="PSUM") as pps_pool,
            tc.tile_pool(name="small", bufs=16) as spool,
            tc.tile_pool(name="outp", bufs=12) as opool,
        ):
            # Constants ride the ACT HWDGE ring so xp loads own the SP ring.
            # ident32 first: it gates the main pipeline's transposes.
            ident32_sb = cpool.tile([128, 128], tdt, tag="ident32")
            nc.scalar.dma_start(ident32_sb[:], ident32)
            w_sb = cpool.tile([D, L], f16, tag="w")
            nc.scalar.dma_start(w_sb[:], w)
            ident_sb = cpool.tile([128, 128], f16, tag="ident")
            nc.scalar.dma_start(ident_sb[:], ident)
            bb_sb = None
            if has_bias:
                bb_sb = cpool.tile([128, 2 * D], f16, tag="bb")
                nc.scalar.dma_start(bb_sb[:], bb)

            copy_fn = mybir.ActivationFunctionType.Copy

            def alpha_from_pt(pt_ps, p_cnt, g_cnt):
                # q = 1 + p on the ACT engine (reads PSUM, frees DVE cycles),
                # then product over the 4 layers; fp16 result.
                q_sb = spool.tile([p_cnt, L * g_cnt], f32, tag="q")
                nc.scalar.activation(q_sb[:], pt_ps[:], copy_fn, bias=1.0)
                a16 = spool.tile([p_cnt, g_cnt], f16, tag="a16")
                if has_bias:
                    qv = q_sb[:].rearrange("p (g l) -> p g l", l=L)
                    a = spool.tile([p_cnt, g_cnt], f32, tag="a0")
                    # c_0 == 0 always (beta_0 = 0)
                    nc.vector.tensor_copy(a[:], qv[:, :, 0])
                    for l in range(1, L):
                        last = l == L - 1
                        t = spool.tile([p_cnt, g_cnt], f32 if not last else f32,
                                       tag=f"a{l}")
                        nc.vector.tensor_mul(t[:], a[:], qv[:, :, l])
                        if cs[l] != 0.0:
                            t2 = spool.tile([p_cnt, g_cnt], f32, tag=f"ac{l}")
                            nc.vector.tensor_scalar_add(t2[:], t[:], float(cs[l]))
                            t = t2
                        a = t
                    nc.vector.tensor_copy(a16[:], a[:])
                else:
                    # t[g, u] = q[g, 2u] * q[g, 2u+1], then a = t[:,0]*t[:,1]
                    qp = q_sb[:].rearrange("p (g u l) -> p g u l", u=2, l=2)
                    t = spool.tile([p_cnt, 2 * g_cnt], f32, tag="a1")
                    tv = t[:].rearrange("p (g u) -> p g u", u=2)
                    nc.vector.tensor_mul(tv, qp[:, :, :, 0], qp[:, :, :, 1])
                    nc.vector.tensor_mul(a16[:], tv[:, :, 0], tv[:, :, 1])
                return a16

            def project_st(xp32, pt_slice):
                # xp32: [128, 512] f32 view of this supertile's packed words.
                # --- packed PE transposes (2 fp16 chunks per transpose) ---
                # All 4 pairs land in ONE PSUM bank; one ACT copy per
                # supertile.  Cross-supertile overlap comes from tps bufs.
                xt_ps = tps_pool.tile([128, NPAIR * D], tdt, tag="xtps")
                xt_sb = xtpool.tile([128, NPAIR * D], tdt, tag="xt")
                for j in range(NPAIR):
                    nc.tensor.transpose(
                        xt_ps[:, j * D : (j + 1) * D],
                        xp32[:, j * D : (j + 1) * D],
                        ident32_sb[:],
                    )
                nc.scalar.copy(xt_sb[:], xt_ps[:])

                # --- projection matmuls: pt[:, g*L:(g+1)*L], g = 2j+q ---
                xt16 = xt_sb[:].bitcast(f16).rearrange("d (j b q) -> d j b q", b=D, q=2)
                for g in range(G):
                    j, qq = g // 2, g % 2
                    nc.tensor.matmul(
                        pt_slice[:, g * L : (g + 1) * L],
                        lhsT=xt16[:, j, :, qq],
                        rhs=w_sb[:],
                        start=True,
                        stop=True,
                    )

            NB = 4  # supertiles per store batch (1 MB stores)
            pair_tile = [None]

            def emit_final(s, xp32, a16):
                # Packed-layout multiply: every operand fp16 innermost step 1
                # -> DVE dual-pump.  alpha pair [a1|a0] re-read across d.
                # NB supertiles share one output tile so stores are 1 MB
                # (8 KB/partition descriptors, 1/NB the SWDGE descgen work).
                half = s % NB
                if half == 0:
                    pair_tile[0] = opool.tile(
                        [128, NB * G * D], f16, name="opair", tag="o"
                    )
                out_sb = pair_tile[0]
                base = half * G * D
                ov4 = out_sb[:, base : base + G * D].rearrange(
                    "p (j d q) -> p j d q", d=D, q=2
                )
                xv4 = xp32.bitcast(f16).rearrange("p (j d q) -> p j d q", d=D, q=2)
                av4 = (
                    a16[:]
                    .rearrange("p (j q) -> p j q", q=2)
                    .unsqueeze(2)
                    .broadcast_to([128, NPAIR, D, 2])
                )
                if has_bias:
                    t_sb = opool.tile([128, G * D], f16, tag="t")
                    tv4 = t_sb[:].rearrange("p (j d q) -> p j d q", d=D, q=2)
                    bv4 = (
                        bb_sb[:]
                        .rearrange("p (d q) -> p d q", q=2)
                        .unsqueeze(1)
                        .broadcast_to([128, NPAIR, D, 2])
                    )
                    nc.vector.tensor_mul(tv4, xv4, av4)
                    nc.vector.tensor_add(ov4, tv4, bv4)
                else:
                    nc.vector.tensor_mul(ov4, xv4, av4)
                if half == NB - 1 or s == NSUP - 1:
                    n = (half + 1) * G * D
                    s0 = s - half
                    nc.gpsimd.dma_start(
                        out[:, s0 * G * D : s0 * G * D + n], out_sb[:, :n]
                    )

            def block_rem():
                p_cnt = REM
                x_sb = xpool.tile([p_cnt, D], f16, tag="xr")
                # scalar ring: keep the SP ring exclusively for x loads
                nc.scalar.dma_start(x_sb[:], xrem)
                xt_ps = tps_pool.tile([128, p_cnt], f16, tag="xtps")
                xt_sb = xtpool.tile([128, p_cnt], f16, tag="xtr")
                pt_ps = pps_pool.tile([p_cnt, L], f32, tag="pt")
                nc.tensor.transpose(xt_ps[:], x_sb[:], ident_sb[:p_cnt, :p_cnt])
                nc.scalar.copy(xt_sb[:], xt_ps[:])
                nc.tensor.matmul(
                    pt_ps[:], lhsT=xt_sb[:], rhs=w_sb[:], start=True, stop=True
                )
                a16 = alpha_from_pt(pt_ps, p_cnt, 1)
                out_sb = opool.tile([p_cnt, D], f16, tag="or")
                if has_bias:
                    t_sb = opool.tile([p_cnt, D], f16, tag="tr")
                    nc.vector.tensor_mul(
                        t_sb[:].rearrange("p (u d) -> p u d", u=1),
                        x_sb[:].rearrange("p (u d) -> p u d", u=1),
                        a16[:].to_broadcast([p_cnt, 1, D]),
                    )
                    bv = bb_sb[:p_cnt].rearrange("p (d q) -> p d q", q=2)[:, :, 0]
                    nc.vector.tensor_add(out_sb[:], t_sb[:], bv)
                else:
                    nc.vector.tensor_mul(
                        out_sb[:].rearrange("p (u d) -> p u d", u=1),
                        x_sb[:].rearrange("p (u d) -> p u d", u=1),
                        a16[:].to_broadcast([p_cnt, 1, D]),
                    )
                # SWDGE: a sync-ring store would head-of-line block the
                # group loads behind it in the HWDGE FIFO (measured +4.9 us).
                nc.gpsimd.dma_start(outr, out_sb[:])

            # Projections run per supertile; the alpha chain is batched over
            # PAIRS of supertiles (one pt tile, one ACT +1, two DVE muls) to
            # halve DVE small-op overhead.  Finished (s, x, alpha) tuples
            # queue in `flights`; emits trail by ~2 supertiles so the DVE
            # multiply/store cadence stays smooth.
            pair = []     # [(s, xp32), ...] accumulating to 2
            flights = []  # [(s, xp32, a16slice), ...] awaiting emit

            def flush_pair():
                if not pair:
                    return
                n = len(pair)
                pt = pps_pool.tile([128, n * L * G], f32, name="pt", tag="pt")
                for i, (ss, xv) in enumerate(pair):
                    project_st(xv, pt[:, i * L * G : (i + 1) * L * G])
                a16 = alpha_from_pt(pt, 128, n * G)
                for i, (ss, xv) in enumerate(pair):
                    flights.append((ss, xv, a16[:, i * G : (i + 1) * G]))
                pair.clear()

            s = 0
            rem_done = not REM
            for gsz in GROUPS:
                gt = xpool.tile([128, gsz * WPS], tdt, tag="x")
                nc.sync.dma_start(gt[:], xp[:, s * WPS : (s + gsz) * WPS])
                for ls in range(gsz):
                    pair.append((s + ls, gt[:, ls * WPS : (ls + 1) * WPS]))
                    if len(pair) == 2:
                        flush_pair()
                    while len(flights) > 2:
                        emit_final(*flights.pop(0))
                s += gsz
                if not rem_done and s >= 8:
                    # Mid-stream: independent work, fills scheduling slack
                    # without delaying the first loads or the kernel tail.
                    block_rem()
                    rem_done = True
            flush_pair()
            while flights:
                emit_final(*flights.pop(0))

    nc.compile()
    return nc


def _pack_shard(xs):
    # xs: [ROWS, D] float32 -> fp16 packed partition-major [128, NSUP*512] f32
    # words; word (p, s, j, d) = [x(s*1024+p*8+2j+1, d) | x(s*1024+p*8+2j, d)].
    x16 = xs[: NSUP * SUP].astype(np.float16).reshape(NSUP, 128, NPAIR, 2, D)
    # -> (p, s, j, d, q)
    pk = np.ascontiguousarray(x16.transpose(1, 0, 2, 4, 3)).reshape(128, -1)
    return pk.view(np.float32)


def _unpack_out(o, orem):
    # o: [128, NSUP*1024] f16 packed (s, j, d, q) per partition -> [ROWS, D] f32
    o5 = o.reshape(128, NSUP, NPAIR, D, 2).transpose(1, 0, 2, 4, 3)
    main = np.ascontiguousarray(o5).reshape(NSUP * SUP, D)
    full = np.empty((ROWS, D), dtype=np.float32)
    full[: NSUP * SUP] = main
    if REM:
        full[NSUP * SUP :] = orem
    return full


def kernel(inputs, kernels, biases):
    global LAST_RESULTS
    import os

    if os.environ.get("BASS_TRACE"):
        # run_bass_kernel_spmd's trace path hard-imports antenv.axon_hooks,
        # which not every image ships; fall back to no-trace instead of
        # crashing when it is absent.
        try:
            import antenv.axon_hooks  # noqa: F401
        except ImportError:
            os.environ["BASS_NEVER_TRACE"] = "1"

    from concourse.bass_utils import run_bass_kernel_spmd

    x = np.ascontiguousarray(np.asarray(inputs), dtype=np.float32)
    assert x.shape == (B, D), x.shape
    kern = np.asarray(kernels, dtype=np.float32).reshape(L, D)
    bias = np.asarray(biases, dtype=np.float32).reshape(L, D)

    W = np.ascontiguousarray(kern.T)  # [D, L]
    has_bias = bool(np.any(bias))
    cs = []
    beta = np.zeros(D, dtype=np.float32)
    for l in range(L):
        cs.append(float(np.dot(beta.astype(np.float64), kern[l].astype(np.float64))))
        beta = beta + bias[l]

    key = (has_bias, tuple(cs) if has_bias else None)
    nc = _CACHE.get(key)
    if nc is None:
        nc = _build(cs, has_bias)
        _CACHE[key] = nc

    in_maps = []
    for i in range(N_CORES):
        xs = x[i * ROWS : (i + 1) * ROWS]
        m = {
            "xp": _pack_shard(xs),
            "w": W.astype(np.float16),
            "ident": np.eye(128, dtype=np.float16),
            "ident32": np.eye(128, dtype=np.float32),
        }
        if REM:
            m["xrem"] = xs[NSUP * SUP :].astype(np.float16)
        if has_bias:
            # bb[p, (d q)] = beta[d] for both q halves.
            bb16 = np.broadcast_to(
                beta.astype(np.float16)[None, :, None], (128, D, 2)
            )
            m["bb"] = np.ascontiguousarray(bb16).reshape(128, 2 * D)
        in_maps.append(m)

    res = run_bass_kernel_spmd(nc, in_maps, core_ids=list(range(N_CORES)))
    LAST_RESULTS = res
    outs = []
    for i in range(N_CORES):
        o = res.results[i]["out"]
        orem = res.results[i]["outr"] if REM else None
        outs.append(_unpack_out(o, orem))
    return np.concatenate(outs, axis=0).astype(np.float32)
